# revision 10
# baseline (speedup 1.0000x reference)
"""Trainium2 Bass kernel for nn_ByteEmbedding (segment_reduce).

Computation (per batch row, one row per NeuronCore, 8 cores):
  byte_emb = emb_weight[x] * sqrt(128)            # gather  [8192, 128]
  grouped  = segment_mean(byte_emb, byte_groups)  # ragged  [2048, 128]
  out      = grouped @ out_proj_w.T               # proj    [2048, 1024]

v2 pipeline (transposed layout [dim, pos], 16 groups of 128 tokens):
  1. x uploaded once as [1, S] int16; partition-broadcast DMA replicates it
     to [128, W] per quarter-window (16 KB HBM traffic instead of 2 MB).
  2. One-hot vocab rows XohT[v, i] = (x[i] == v) built on DVE in bf16
     (int16-in/bf16-out hits the 4x DVE mode; 3 chunks of 128 vocab rows,
     one set per quarter-window).
  3. byte_emb^T = (E*sqrt(128))^T @ XohT as bf16 matmuls on the PE
     (1 cycle/row vs 4 for fp32), accumulated per group window in PSUM.
  4. Exclusive prefix sums per group window (DVE tensor_tensor_scan, f32,
     read straight from PSUM); segment sums are differences of the scan at
     host-precomputed boundary positions (gpsimd ap_gather).
  5. Mean via reciprocal counts folded into the mandatory PSUM->SBUF copy
     after the projection (ACT scalar.mul with per-partition scale).
  6. out = grp^T.T @ W^T per 128-token group in bf16; W^T is uploaded
     host-transposed and cast to bf16 on device.
  7. Output DMAs alternate between the SP and Activation HW queues.

The group windows are the union over the 8 rows, so one SPMD program
serves all cores; per-core behavior enters only through uploaded integer
index tensors.
"""

import os
import sys

import numpy as np

for _p in ("/opt/trn_rl_repo",):
    if _p not in sys.path and os.path.isdir(_p):
        sys.path.append(_p)

import concourse.bacc as bacc
import concourse.bass as bass
import concourse.mybir as mybir
import concourse.tile as tile
from concourse.bass_utils import run_bass_kernel_spmd

B = 8
S = 8192          # bytes per row
V = 384           # vocab (= 3 * 128)
D = 128           # byte dim
E = 1024          # out dim
T = 2048          # tokens
P = 128
NGRP = 16         # token groups of 128
TG = T // NGRP    # 128 tokens per group
NB = TG + 1       # boundaries per group (inclusive)
NBPAD = 144       # padded boundary count (16*9, %4==0)
NBSLOT = 16       # idx words per group slot (32-byte aligned for ap_gather)
WCAP = 1024       # max positions per group window (psum: 2 banks)
SL = WCAP + 1     # scan tile length
SCALE = float(D) ** 0.5
dt = mybir.dt
F32 = dt.float32
BF16 = dt.bfloat16
ADD = mybir.AluOpType.add


def _windows(starts):
    """Union [lo, hi) position window per group / quarter over all rows."""
    w = []
    for g in range(NGRP):
        lo = int(starts[:, TG * g].min())
        hi = int(starts[:, TG * (g + 1)].max())
        w.append((lo, hi))
    return w


def _build(windows) -> bacc.Bacc:
    nc = bacc.Bacc(
        "TRN2",
        target_bir_lowering=False,
        debug=False,
        enable_asserts=True,
        num_devices=B,
    )

    x1 = nc.dram_tensor("x1", [1, S], dt.int16, kind="ExternalInput")
    st_a = nc.dram_tensor("st_a", [P, NGRP], dt.int32, kind="ExternalInput")
    st_b = nc.dram_tensor("st_b", [P, NGRP], dt.int32, kind="ExternalInput")
    bidx = nc.dram_tensor("bidx", [P, NGRP * NBSLOT], dt.int16,
                          kind="ExternalInput")
    vcol = nc.dram_tensor("vcol", [P, V // P], F32, kind="ExternalInput")
    emb_weight = nc.dram_tensor("emb_weight", [V, D], F32, kind="ExternalInput")
    wt = nc.dram_tensor("wt", [D, E], F32, kind="ExternalInput")  # host-transposed
    out = nc.dram_tensor("out", [T, E], F32, kind="ExternalOutput")

    # quarter compare-windows (4 groups each)
    NQ = 4
    GPQ = NGRP // NQ
    qwin = []
    for q in range(NQ):
        qwin.append((windows[GPQ * q][0], windows[GPQ * q + GPQ - 1][1]))

    with tile.TileContext(nc) as tc:
        with (
            tc.tile_pool(name="cst", bufs=1) as cst,
            tc.tile_pool(name="xq", bufs=1) as xq_pool,
            tc.tile_pool(name="oh", bufs=1) as oh_pool,
            tc.tile_pool(name="work", bufs=1) as work,
            tc.tile_pool(name="ps_g", bufs=1, space="PSUM") as ps_g,
            tc.tile_pool(name="ps_o", bufs=1, space="PSUM") as ps_o,
        ):
            # vcol first on the fast Sync queue: it gates the first compare
            vcol_t = cst.tile([P, V // P], F32, name="vcol_t")
            nc.sync.dma_start(out=vcol_t[:], in_=vcol.ap())

            # ---- x quarter-window broadcasts (Sync queue) ----
            xq = []
            for q in range(NQ):
                lo, hi = qwin[q]
                xt = xq_pool.tile([P, hi - lo], dt.int16, name=f"xq{q}",
                                  tag="xq", bufs=NQ)
                nc.sync.dma_start(out=xt[:], in_=x1.ap()[0:1, lo:hi].to_broadcast(
                    [P, hi - lo]))
                xq.append(xt)

            # ---- prologue: constants / weights ----
            zcol = cst.tile([P, 1], F32, name="zcol")
            nc.vector.memset(zcol[:], 0.0)

            # emb chunks + bf16 prep on ACT (needed by first emb matmul)
            emb_f = []
            for v in range(V // P):
                ef = cst.tile([P, D], F32, name=f"emb_f{v}")
                nc.scalar.dma_start(out=ef[:], in_=emb_weight.ap()[v * P:(v + 1) * P, :])
                emb_f.append(ef)
            embb = []
            for v in range(V // P):
                eb = cst.tile([P, D], BF16, name=f"embb{v}")
                nc.scalar.mul(eb[:], emb_f[v][:], SCALE)
                embb.append(eb)

            wt_f = cst.tile([P, E], F32, name="wt_f")
            nc.scalar.dma_start(out=wt_f[:], in_=wt.ap())
            wtb = cst.tile([P, E], BF16, name="wtb")
            nc.scalar.copy(wtb[:], wt_f[:])

            # small index tensors on gpsimd SWDGE
            bidx_t = cst.tile([P, NGRP * NBSLOT], dt.int16, name="bidx_t")
            nc.gpsimd.dma_start(out=bidx_t[:], in_=bidx.ap())
            sta_i = cst.tile([P, NGRP], dt.int32, name="sta_i")
            nc.gpsimd.dma_start(out=sta_i[:], in_=st_a.ap())
            stb_i = cst.tile([P, NGRP], dt.int32, name="stb_i")
            nc.gpsimd.dma_start(out=stb_i[:], in_=st_b.ap())

            # recip counts (DVE, small)
            sta_f = cst.tile([P, NGRP], F32, name="sta_f")
            nc.vector.tensor_copy(out=sta_f[:], in_=sta_i[:])
            stb_f = cst.tile([P, NGRP], F32, name="stb_f")
            nc.vector.tensor_copy(out=stb_f[:], in_=stb_i[:])
            cnt = cst.tile([P, NGRP], F32, name="cnt")
            nc.vector.tensor_tensor(out=cnt[:], in0=stb_f[:], in1=sta_f[:],
                                    op=mybir.AluOpType.subtract)
            nc.vector.tensor_scalar(out=cnt[:], in0=cnt[:], scalar1=1.0,
                                    scalar2=None, op0=mybir.AluOpType.max)
            recip = cst.tile([P, NGRP], F32, name="recip")
            nc.vector.reciprocal(out=recip[:], in_=cnt[:])

            # ---- software pipeline over 16 groups ----
            # per step g: compares (if new quarter), emb matmuls g, scan g,
            # gather g-1, diff g-2, proj g-3, scale-copy g-3, out dma g-3
            # (diff lags the gather by one step so the gpsimd gather's launch
            # latency overlaps the next scan instead of stalling DVE)
            # pre-zero col0 of the scg ring buffers (scan writes [:, 1:] only)
            for _i in range(3):
                sc0 = work.tile([P, SL], F32, name="scg", tag="scg", bufs=3)
                nc.vector.memset(sc0[:, 0:1], 0.0)

            ohs = [None] * NQ          # live one-hot tiles per quarter
            psg = [None] * NGRP
            scg = [None] * NGRP
            bnd = [None] * NGRP
            grp = [None] * NGRP
            pso = [None] * NGRP
            osb = [None] * NGRP

            def emit_compare(q, v):
                lo, hi = qwin[q]
                if ohs[q] is None:
                    ohs[q] = [None] * (V // P)
                oh = oh_pool.tile([P, hi - lo], BF16, name=f"oh{v}",
                                  tag=f"oh{v}", bufs=2)
                nc.vector.tensor_scalar(
                    out=oh[:], in0=xq[q][:], scalar1=vcol_t[:, v:v + 1],
                    scalar2=None, op0=mybir.AluOpType.is_equal)
                ohs[q][v] = oh

            def emit_emb(g):
                q = g // GPQ
                qlo = qwin[q][0]
                lo, hi = windows[g]
                L = hi - lo
                pg = ps_g.tile([P, WCAP], F32, name="psg", tag="psg", bufs=3)
                nsub = (L + 511) // 512
                for s in range(nsub):
                    c0, c1 = 512 * s, min(512 * (s + 1), L)
                    for v in range(V // P):
                        nc.tensor.matmul(
                            out=pg[:, c0:c1], lhsT=embb[v][:],
                            rhs=ohs[q][v][:, lo - qlo + c0:lo - qlo + c1],
                            start=(v == 0), stop=(v == V // P - 1))
                psg[g] = pg

            def emit_scan(g):
                L = windows[g][1] - windows[g][0]
                sc = work.tile([P, SL], F32, name="scg", tag="scg", bufs=3)
                nc.vector.tensor_tensor_scan(
                    out=sc[:, 1:1 + L], data0=psg[g][:, 0:L],
                    data1=zcol[:].to_broadcast([P, L]),
                    initial=0.0, op0=ADD, op1=ADD)
                scg[g] = sc

            def emit_gather(g):
                bt = work.tile([P, NBPAD], F32, name="bnd", tag="bnd", bufs=4)
                nc.gpsimd.ap_gather(
                    out_ap=bt[:], in_ap=scg[g][:],
                    idxs_ap=bidx_t[:, g * NBSLOT:g * NBSLOT + NBPAD // 16],
                    channels=P, num_elems=SL, d=1, num_idxs=NBPAD)
                bnd[g] = bt

            def emit_diff(g):
                gt = work.tile([P, TG], BF16, name="grp", tag="grp", bufs=3)
                nc.vector.tensor_tensor(
                    out=gt[:], in0=bnd[g][:, 1:NB], in1=bnd[g][:, 0:NB - 1],
                    op=mybir.AluOpType.subtract)
                grp[g] = gt

            def emit_proj(g):
                halves = []
                for h in range(2):
                    po = ps_o.tile([P, E // 2], F32, name="pso", tag="pso", bufs=2)
                    nc.tensor.matmul(
                        out=po[:], lhsT=grp[g][:],
                        rhs=wtb[:, h * 512:(h + 1) * 512],
                        start=True, stop=True)
                    halves.append(po)
                pso[g] = halves

            def emit_out(g):
                ot = work.tile([P, E], F32, name="osb", tag="osb", bufs=3)
                for h in range(2):
                    nc.scalar.mul(ot[:, h * 512:(h + 1) * 512], pso[g][h][:],
                                  recip[:, g:g + 1])
                nc.sync.dma_start(out=out.ap()[g * TG:(g + 1) * TG, :], in_=ot[:])
                osb[g] = ot

            for step in range(NGRP + 4):
                g = step
                if 4 <= g <= NGRP + 3:
                    emit_proj(g - 4)
                if g < NGRP:
                    if g == 0:
                        for v in range(V // P):
                            emit_compare(0, v)
                    qn, ph = g // GPQ + 1, g % GPQ
                    if qn < NQ and 1 <= ph <= V // P:
                        emit_compare(qn, ph - 1)
                    emit_emb(g)
                    emit_scan(g)
                if 1 <= g <= NGRP:
                    emit_gather(g - 1)
                if 3 <= g <= NGRP + 2:
                    emit_diff(g - 3)
                if 4 <= g <= NGRP + 3:
                    emit_out(g - 4)

    nc.compile()
    return nc


def _prep_inputs(x, byte_groups, emb_weight, out_proj_w, windows, starts):
    """Host-side integer index plumbing + weight layout prep."""
    wt_np = np.ascontiguousarray(np.asarray(out_proj_w, np.float32).T)  # [128,1024]
    emb_np = np.ascontiguousarray(np.asarray(emb_weight, np.float32))
    vcol_np = np.zeros((P, V // P), np.float32)
    for v in range(V // P):
        vcol_np[:, v] = v * P + np.arange(P)

    in_maps = []
    for k in range(B):
        sta = starts[k, :T].reshape(NGRP, TG).transpose(1, 0).astype(np.int32)
        stb = starts[k, 1:T + 1].reshape(NGRP, TG).transpose(1, 0).astype(np.int32)
        # boundary indices per group, wrapped in 16 partitions, x8 replicated
        bx = np.zeros((P, NGRP * NBSLOT), np.int16)
        for g in range(NGRP):
            lo = windows[g][0]
            loc = (starts[k, TG * g:TG * (g + 1) + 1] - lo).astype(np.int16)
            pad = np.full(NBPAD, loc[-1], np.int16)
            pad[:NB] = loc
            w = pad.reshape(NBPAD // 16, 16).T  # [16, 9]
            for rep in range(8):
                bx[16 * rep:16 * (rep + 1),
                   g * NBSLOT:g * NBSLOT + NBPAD // 16] = w
        in_maps.append({
            "x1": x[k].astype(np.int16).reshape(1, S),
            "st_a": np.ascontiguousarray(sta),
            "st_b": np.ascontiguousarray(stb),
            "bidx": bx,
            "vcol": vcol_np,
            "emb_weight": emb_np,
            "wt": wt_np,
        })
    return in_maps


def _run(x, byte_groups, emb_weight, out_proj_w, trace=False, **kw):
    x = np.asarray(x)
    byte_groups = np.asarray(byte_groups)
    starts = np.stack(
        [np.searchsorted(byte_groups[k], np.arange(T + 1)) for k in range(B)]
    )
    windows = _windows(starts)
    assert max(hi - lo for lo, hi in windows) <= WCAP, windows
    nc = _build(windows)
    in_maps = _prep_inputs(x, byte_groups, emb_weight, out_proj_w, windows, starts)
    res = run_bass_kernel_spmd(nc, in_maps, core_ids=list(range(B)), trace=trace, **kw)
    outs = np.stack([res.results[k]["out"] for k in range(B)], axis=0)
    return outs, res


def kernel(x, byte_groups, emb_weight, out_proj_w):
    outs, _ = _run(x, byte_groups, emb_weight, out_proj_w, trace=False)
    return outs


# revision 12
# speedup vs baseline: 1.0543x; 1.0543x over previous
"""Trainium2 Bass kernel for nn_ByteEmbedding (segment_reduce).

Computation (per batch row, one row per NeuronCore, 8 cores):
  byte_emb = emb_weight[x] * sqrt(128)            # gather  [8192, 128]
  grouped  = segment_mean(byte_emb, byte_groups)  # ragged  [2048, 128]
  out      = grouped @ out_proj_w.T               # proj    [2048, 1024]

v2 pipeline (transposed layout [dim, pos], 16 groups of 128 tokens):
  1. x uploaded once as [1, S] int16; partition-broadcast DMA replicates it
     to [128, W] per quarter-window (16 KB HBM traffic instead of 2 MB).
  2. One-hot vocab rows XohT[v, i] = (x[i] == v) built on DVE in bf16
     (int16-in/bf16-out hits the 4x DVE mode; 3 chunks of 128 vocab rows,
     one set per quarter-window).
  3. byte_emb^T = (E*sqrt(128))^T @ XohT as bf16 matmuls on the PE
     (1 cycle/row vs 4 for fp32), accumulated per group window in PSUM.
  4. Exclusive prefix sums per group window (DVE tensor_tensor_scan, f32,
     read straight from PSUM); segment sums are differences of the scan at
     host-precomputed boundary positions (gpsimd ap_gather).
  5. Mean via reciprocal counts folded into the mandatory PSUM->SBUF copy
     after the projection (ACT scalar.mul with per-partition scale).
  6. out = grp^T.T @ W^T per 128-token group in bf16; W^T is uploaded
     host-transposed and cast to bf16 on device.
  7. Output DMAs alternate between the SP and Activation HW queues.

The group windows are the union over the 8 rows, so one SPMD program
serves all cores; per-core behavior enters only through uploaded integer
index tensors.
"""

import os
import sys

import numpy as np

for _p in ("/opt/trn_rl_repo",):
    if _p not in sys.path and os.path.isdir(_p):
        sys.path.append(_p)

import concourse.bacc as bacc
import concourse.bass as bass
import concourse.mybir as mybir
import concourse.tile as tile
from concourse.bass_utils import run_bass_kernel_spmd

B = 8
S = 8192          # bytes per row
V = 384           # vocab (= 3 * 128)
D = 128           # byte dim
E = 1024          # out dim
T = 2048          # tokens
P = 128
NGRP = 8          # token groups of 256
TG = T // NGRP    # 256 tokens per group
NB = TG + 1       # boundaries per group (inclusive)
NBPAD = 272       # padded boundary count (16*17, %4==0)
NBSLOT = 32       # idx words per group slot (32-byte aligned for ap_gather)
WCAP = 1280       # max positions per group window (psum: 3 banks)
SL = WCAP + 1     # scan tile length
SCALE = float(D) ** 0.5
dt = mybir.dt
F32 = dt.float32
BF16 = dt.bfloat16
ADD = mybir.AluOpType.add


def _windows(starts):
    """Union [lo, hi) position window per group / quarter over all rows."""
    w = []
    for g in range(NGRP):
        lo = int(starts[:, TG * g].min())
        hi = int(starts[:, TG * (g + 1)].max())
        w.append((lo, hi))
    return w


def _build(windows) -> bacc.Bacc:
    nc = bacc.Bacc(
        "TRN2",
        target_bir_lowering=False,
        debug=False,
        enable_asserts=True,
        num_devices=B,
    )

    x1 = nc.dram_tensor("x1", [1, S], dt.int16, kind="ExternalInput")
    st_a = nc.dram_tensor("st_a", [P, T // P], dt.int32, kind="ExternalInput")
    st_b = nc.dram_tensor("st_b", [P, T // P], dt.int32, kind="ExternalInput")
    bidx = nc.dram_tensor("bidx", [P, NGRP * NBSLOT], dt.int16,
                          kind="ExternalInput")
    vcol = nc.dram_tensor("vcol", [P, V // P], F32, kind="ExternalInput")
    emb_weight = nc.dram_tensor("emb_weight", [V, D], F32, kind="ExternalInput")
    wt = nc.dram_tensor("wt", [D, E], F32, kind="ExternalInput")  # host-transposed
    out = nc.dram_tensor("out", [T, E], F32, kind="ExternalOutput")

    # compare-window spans (4 groups each)
    NQ = 2
    GPQ = NGRP // NQ
    qwin = []
    for q in range(NQ):
        qwin.append((windows[GPQ * q][0], windows[GPQ * q + GPQ - 1][1]))

    with tile.TileContext(nc) as tc:
        with (
            tc.tile_pool(name="cst", bufs=1) as cst,
            tc.tile_pool(name="xq", bufs=1) as xq_pool,
            tc.tile_pool(name="oh", bufs=1) as oh_pool,
            tc.tile_pool(name="work", bufs=1) as work,
            tc.tile_pool(name="ps_g", bufs=1, space="PSUM") as ps_g,
            tc.tile_pool(name="ps_o", bufs=1, space="PSUM") as ps_o,
        ):
            # vcol first on the fast Sync queue: it gates the first compare
            vcol_t = cst.tile([P, V // P], F32, name="vcol_t")
            nc.sync.dma_start(out=vcol_t[:], in_=vcol.ap())

            # ---- x quarter-window broadcasts (Sync queue) ----
            xq = []
            for q in range(NQ):
                lo, hi = qwin[q]
                xt = xq_pool.tile([P, hi - lo], dt.int16, name=f"xq{q}",
                                  tag="xq", bufs=NQ)
                nc.sync.dma_start(out=xt[:], in_=x1.ap()[0:1, lo:hi].to_broadcast(
                    [P, hi - lo]))
                xq.append(xt)

            # ---- prologue: constants / weights ----
            zcol = cst.tile([P, 1], F32, name="zcol")
            nc.vector.memset(zcol[:], 0.0)

            # emb chunks + bf16 prep on ACT (needed by first emb matmul)
            emb_f = []
            for v in range(V // P):
                ef = cst.tile([P, D], F32, name=f"emb_f{v}")
                nc.scalar.dma_start(out=ef[:], in_=emb_weight.ap()[v * P:(v + 1) * P, :])
                emb_f.append(ef)
            embb = []
            for v in range(V // P):
                eb = cst.tile([P, D], BF16, name=f"embb{v}")
                nc.scalar.mul(eb[:], emb_f[v][:], SCALE)
                embb.append(eb)

            wt_f = cst.tile([P, E], F32, name="wt_f")
            nc.scalar.dma_start(out=wt_f[:], in_=wt.ap())
            wtb = cst.tile([P, E], BF16, name="wtb")
            nc.scalar.copy(wtb[:], wt_f[:])

            # small index tensors on gpsimd SWDGE
            bidx_t = cst.tile([P, NGRP * NBSLOT], dt.int16, name="bidx_t")
            nc.gpsimd.dma_start(out=bidx_t[:], in_=bidx.ap())
            sta_i = cst.tile([P, T // P], dt.int32, name="sta_i")
            nc.gpsimd.dma_start(out=sta_i[:], in_=st_a.ap())
            stb_i = cst.tile([P, T // P], dt.int32, name="stb_i")
            nc.gpsimd.dma_start(out=stb_i[:], in_=st_b.ap())

            # recip counts (DVE, small)
            sta_f = cst.tile([P, T // P], F32, name="sta_f")
            nc.vector.tensor_copy(out=sta_f[:], in_=sta_i[:])
            stb_f = cst.tile([P, T // P], F32, name="stb_f")
            nc.vector.tensor_copy(out=stb_f[:], in_=stb_i[:])
            cnt = cst.tile([P, T // P], F32, name="cnt")
            nc.vector.tensor_tensor(out=cnt[:], in0=stb_f[:], in1=sta_f[:],
                                    op=mybir.AluOpType.subtract)
            nc.vector.tensor_scalar(out=cnt[:], in0=cnt[:], scalar1=1.0,
                                    scalar2=None, op0=mybir.AluOpType.max)
            recip = cst.tile([P, T // P], F32, name="recip")
            nc.vector.reciprocal(out=recip[:], in_=cnt[:])

            # ---- software pipeline over 16 groups ----
            # per step g: compares (if new quarter), emb matmuls g, scan g,
            # gather g-1, diff g-2, proj g-3, scale-copy g-3, out dma g-3
            # (diff lags the gather by one step so the gpsimd gather's launch
            # latency overlaps the next scan instead of stalling DVE)
            # pre-zero col0 of the scg ring buffers (scan writes [:, 1:] only)
            for _i in range(3):
                sc0 = work.tile([P, SL], F32, name="scg", tag="scg", bufs=3)
                nc.vector.memset(sc0[:, 0:1], 0.0)

            ohs = [None] * NQ          # live one-hot tiles per quarter
            psg = [None] * NGRP
            scg = [None] * NGRP
            bnd = [None] * NGRP
            grp = [None] * NGRP
            pso = [None] * NGRP
            osb = [None] * NGRP

            def emit_compares(q):
                lo, hi = qwin[q]
                tiles = []
                for v in range(V // P):
                    oh = oh_pool.tile([P, hi - lo], BF16, name=f"oh{v}",
                                      tag=f"oh{v}", bufs=2)
                    nc.vector.tensor_scalar(
                        out=oh[:], in0=xq[q][:], scalar1=vcol_t[:, v:v + 1],
                        scalar2=None, op0=mybir.AluOpType.is_equal)
                    tiles.append(oh)
                ohs[q] = tiles

            def emit_emb(g):
                q = g // GPQ
                qlo = qwin[q][0]
                lo, hi = windows[g]
                L = hi - lo
                pg = ps_g.tile([P, WCAP], F32, name="psg", tag="psg", bufs=2)
                nsub = (L + 511) // 512
                for s in range(nsub):
                    c0, c1 = 512 * s, min(512 * (s + 1), L)
                    for v in range(V // P):
                        nc.tensor.matmul(
                            out=pg[:, c0:c1], lhsT=embb[v][:],
                            rhs=ohs[q][v][:, lo - qlo + c0:lo - qlo + c1],
                            start=(v == 0), stop=(v == V // P - 1))
                psg[g] = pg

            def emit_scan(g):
                L = windows[g][1] - windows[g][0]
                sc = work.tile([P, SL], F32, name="scg", tag="scg", bufs=3)
                nc.vector.tensor_tensor_scan(
                    out=sc[:, 1:1 + L], data0=psg[g][:, 0:L],
                    data1=zcol[:].to_broadcast([P, L]),
                    initial=0.0, op0=ADD, op1=ADD)
                scg[g] = sc

            def emit_gather(g):
                bt = work.tile([P, NBPAD], F32, name="bnd", tag="bnd", bufs=4)
                nc.gpsimd.ap_gather(
                    out_ap=bt[:], in_ap=scg[g][:],
                    idxs_ap=bidx_t[:, g * NBSLOT:g * NBSLOT + NBPAD // 16],
                    channels=P, num_elems=SL, d=1, num_idxs=NBPAD)
                bnd[g] = bt

            def emit_diff(g):
                gt = work.tile([P, TG], BF16, name="grp", tag="grp", bufs=3)
                nc.vector.tensor_tensor(
                    out=gt[:], in0=bnd[g][:, 1:NB], in1=bnd[g][:, 0:NB - 1],
                    op=mybir.AluOpType.subtract)
                grp[g] = gt

            def emit_proj(g):
                tiles = []
                for j in range(TG // P):
                    for h in range(2):
                        po = ps_o.tile([P, E // 2], F32, name="pso", tag="pso", bufs=2)
                        nc.tensor.matmul(
                            out=po[:], lhsT=grp[g][:, j * P:(j + 1) * P],
                            rhs=wtb[:, h * 512:(h + 1) * 512],
                            start=True, stop=True)
                        tiles.append(po)
                pso[g] = tiles

            def emit_out(g):
                tiles = []
                for j in range(TG // P):
                    r = g * (TG // P) + j
                    ot = work.tile([P, E], F32, name="osb", tag="osb", bufs=3)
                    for h in range(2):
                        nc.scalar.mul(ot[:, h * 512:(h + 1) * 512],
                                      pso[g][2 * j + h][:], recip[:, r:r + 1])
                    nc.sync.dma_start(
                        out=out.ap()[r * P:(r + 1) * P, :], in_=ot[:])
                    tiles.append(ot)
                osb[g] = tiles

            for step in range(NGRP + 4):
                g = step
                if 4 <= g <= NGRP + 3:
                    emit_proj(g - 4)
                if g < NGRP:
                    if g % GPQ == 0:
                        emit_compares(g // GPQ)
                    emit_emb(g)
                    emit_scan(g)
                if 1 <= g <= NGRP:
                    emit_gather(g - 1)
                if 3 <= g <= NGRP + 2:
                    emit_diff(g - 3)
                if 4 <= g <= NGRP + 3:
                    emit_out(g - 4)

    nc.compile()
    return nc


def _prep_inputs(x, byte_groups, emb_weight, out_proj_w, windows, starts):
    """Host-side integer index plumbing + weight layout prep."""
    wt_np = np.ascontiguousarray(np.asarray(out_proj_w, np.float32).T)  # [128,1024]
    emb_np = np.ascontiguousarray(np.asarray(emb_weight, np.float32))
    vcol_np = np.zeros((P, V // P), np.float32)
    for v in range(V // P):
        vcol_np[:, v] = v * P + np.arange(P)

    in_maps = []
    for k in range(B):
        sta = starts[k, :T].reshape(T // P, P).transpose(1, 0).astype(np.int32)
        stb = starts[k, 1:T + 1].reshape(T // P, P).transpose(1, 0).astype(np.int32)
        # boundary indices per group, wrapped in 16 partitions, x8 replicated
        bx = np.zeros((P, NGRP * NBSLOT), np.int16)
        for g in range(NGRP):
            lo = windows[g][0]
            loc = (starts[k, TG * g:TG * (g + 1) + 1] - lo).astype(np.int16)
            pad = np.full(NBPAD, loc[-1], np.int16)
            pad[:NB] = loc
            w = pad.reshape(NBPAD // 16, 16).T  # [16, 9]
            for rep in range(8):
                bx[16 * rep:16 * (rep + 1),
                   g * NBSLOT:g * NBSLOT + NBPAD // 16] = w
        in_maps.append({
            "x1": x[k].astype(np.int16).reshape(1, S),
            "st_a": np.ascontiguousarray(sta),
            "st_b": np.ascontiguousarray(stb),
            "bidx": bx,
            "vcol": vcol_np,
            "emb_weight": emb_np,
            "wt": wt_np,
        })
    return in_maps


def _run(x, byte_groups, emb_weight, out_proj_w, trace=False, **kw):
    x = np.asarray(x)
    byte_groups = np.asarray(byte_groups)
    starts = np.stack(
        [np.searchsorted(byte_groups[k], np.arange(T + 1)) for k in range(B)]
    )
    windows = _windows(starts)
    assert max(hi - lo for lo, hi in windows) <= WCAP, windows
    nc = _build(windows)
    in_maps = _prep_inputs(x, byte_groups, emb_weight, out_proj_w, windows, starts)
    res = run_bass_kernel_spmd(nc, in_maps, core_ids=list(range(B)), trace=trace, **kw)
    outs = np.stack([res.results[k]["out"] for k in range(B)], axis=0)
    return outs, res


def kernel(x, byte_groups, emb_weight, out_proj_w):
    outs, _ = _run(x, byte_groups, emb_weight, out_proj_w, trace=False)
    return outs


# revision 13
# speedup vs baseline: 1.1790x; 1.1182x over previous
"""Trainium2 Bass kernel for nn_ByteEmbedding (segment_reduce).

Computation (per batch row, one row per NeuronCore, 8 cores):
  byte_emb = emb_weight[x] * sqrt(128)            # gather  [8192, 128]
  grouped  = segment_mean(byte_emb, byte_groups)  # ragged  [2048, 128]
  out      = grouped @ out_proj_w.T               # proj    [2048, 1024]

v2 pipeline (transposed layout [dim, pos], 16 groups of 128 tokens):
  1. x uploaded once as [1, S] int16; partition-broadcast DMA replicates it
     to [128, W] per quarter-window (16 KB HBM traffic instead of 2 MB).
  2. One-hot vocab rows XohT[v, i] = (x[i] == v) built on DVE in bf16
     (int16-in/bf16-out hits the 4x DVE mode; 3 chunks of 128 vocab rows,
     one set per quarter-window).
  3. byte_emb^T = (E*sqrt(128))^T @ XohT as bf16 matmuls on the PE
     (1 cycle/row vs 4 for fp32), accumulated per group window in PSUM.
  4. Exclusive prefix sums per group window (DVE tensor_tensor_scan, f32,
     read straight from PSUM); segment sums are differences of the scan at
     host-precomputed boundary positions (gpsimd ap_gather).
  5. Mean via reciprocal counts folded into the mandatory PSUM->SBUF copy
     after the projection (ACT scalar.mul with per-partition scale).
  6. out = grp^T.T @ W^T per 128-token group in bf16; W^T is uploaded
     host-transposed and cast to bf16 on device.
  7. Output DMAs alternate between the SP and Activation HW queues.

The group windows are the union over the 8 rows, so one SPMD program
serves all cores; per-core behavior enters only through uploaded integer
index tensors.
"""

import os
import sys

import numpy as np

for _p in ("/opt/trn_rl_repo",):
    if _p not in sys.path and os.path.isdir(_p):
        sys.path.append(_p)

import concourse.bacc as bacc
import concourse.bass as bass
import concourse.mybir as mybir
import concourse.tile as tile
from concourse.bass_utils import run_bass_kernel_spmd

B = 8
S = 8192          # bytes per row
V = 384           # vocab (= 3 * 128)
D = 128           # byte dim
E = 1024          # out dim
T = 2048          # tokens
P = 128
NGRP = 16         # token groups of 128
TG = T // NGRP    # 128 tokens per group
NB = TG + 1       # boundaries per group (inclusive)
NBPAD = 144       # padded boundary count (16*9, %4==0)
NBSLOT = 16       # idx words per group slot (32-byte aligned for ap_gather)
WCAP = 1024       # max positions per group window (psum: 2 banks)
SL = WCAP + 1     # scan tile length
SCALE = float(D) ** 0.5
dt = mybir.dt
F32 = dt.float32
BF16 = dt.bfloat16
ADD = mybir.AluOpType.add


def _windows(starts):
    """Union [lo, hi) position window per group / quarter over all rows."""
    w = []
    for g in range(NGRP):
        lo = int(starts[:, TG * g].min())
        hi = int(starts[:, TG * (g + 1)].max())
        w.append((lo, hi))
    return w


def _build(windows) -> bacc.Bacc:
    nc = bacc.Bacc(
        "TRN2",
        target_bir_lowering=False,
        debug=False,
        enable_asserts=True,
        num_devices=B,
    )

    x1 = nc.dram_tensor("x1", [1, S], dt.int16, kind="ExternalInput")
    st_a = nc.dram_tensor("st_a", [P, NGRP], dt.int32, kind="ExternalInput")
    st_b = nc.dram_tensor("st_b", [P, NGRP], dt.int32, kind="ExternalInput")
    bidx = nc.dram_tensor("bidx", [P, NGRP * NBSLOT], dt.int16,
                          kind="ExternalInput")
    vcol = nc.dram_tensor("vcol", [P, V // P], F32, kind="ExternalInput")
    emb_weight = nc.dram_tensor("emb_weight", [V, D], F32, kind="ExternalInput")
    wt = nc.dram_tensor("wt", [D, E], F32, kind="ExternalInput")  # host-transposed
    out = nc.dram_tensor("out", [T, E], F32, kind="ExternalOutput")

    # quarter compare-windows (4 groups each)
    NQ = 4
    GPQ = NGRP // NQ
    qwin = []
    for q in range(NQ):
        qwin.append((windows[GPQ * q][0], windows[GPQ * q + GPQ - 1][1]))

    with tile.TileContext(nc) as tc:
        with (
            tc.tile_pool(name="cst", bufs=1) as cst,
            tc.tile_pool(name="xq", bufs=1) as xq_pool,
            tc.tile_pool(name="oh", bufs=1) as oh_pool,
            tc.tile_pool(name="work", bufs=1) as work,
            tc.tile_pool(name="ps_g", bufs=1, space="PSUM") as ps_g,
            tc.tile_pool(name="ps_o", bufs=1, space="PSUM") as ps_o,
        ):
            # vcol first on the fast Sync queue: it gates the first compare
            vcol_t = cst.tile([P, V // P], F32, name="vcol_t")
            nc.sync.dma_start(out=vcol_t[:], in_=vcol.ap())

            # ---- x quarter-window broadcasts (Sync queue) ----
            xq = []
            for q in range(NQ):
                lo, hi = qwin[q]
                xt = xq_pool.tile([P, hi - lo], dt.int16, name=f"xq{q}",
                                  tag="xq", bufs=NQ)
                nc.sync.dma_start(out=xt[:], in_=x1.ap()[0:1, lo:hi].to_broadcast(
                    [P, hi - lo]))
                xq.append(xt)

            # ---- prologue: constants / weights ----
            zcol = cst.tile([P, 1], F32, name="zcol")
            nc.vector.memset(zcol[:], 0.0)

            # emb chunks + bf16 prep on ACT (needed by first emb matmul)
            emb_f = []
            for v in range(V // P):
                ef = cst.tile([P, D], F32, name=f"emb_f{v}")
                nc.scalar.dma_start(out=ef[:], in_=emb_weight.ap()[v * P:(v + 1) * P, :])
                emb_f.append(ef)
            embb = []
            for v in range(V // P):
                eb = cst.tile([P, D], BF16, name=f"embb{v}")
                nc.scalar.mul(eb[:], emb_f[v][:], SCALE)
                embb.append(eb)

            wt_f = cst.tile([P, E], F32, name="wt_f")
            nc.scalar.dma_start(out=wt_f[:], in_=wt.ap())
            wtb = cst.tile([P, E], BF16, name="wtb")
            nc.scalar.copy(wtb[:], wt_f[:])

            # small index tensors on gpsimd SWDGE
            bidx_t = cst.tile([P, NGRP * NBSLOT], dt.int16, name="bidx_t")
            nc.gpsimd.dma_start(out=bidx_t[:], in_=bidx.ap())
            sta_i = cst.tile([P, NGRP], dt.int32, name="sta_i")
            nc.gpsimd.dma_start(out=sta_i[:], in_=st_a.ap())
            stb_i = cst.tile([P, NGRP], dt.int32, name="stb_i")
            nc.gpsimd.dma_start(out=stb_i[:], in_=st_b.ap())

            # recip counts (DVE, small)
            sta_f = cst.tile([P, NGRP], F32, name="sta_f")
            nc.vector.tensor_copy(out=sta_f[:], in_=sta_i[:])
            stb_f = cst.tile([P, NGRP], F32, name="stb_f")
            nc.vector.tensor_copy(out=stb_f[:], in_=stb_i[:])
            cnt = cst.tile([P, NGRP], F32, name="cnt")
            nc.vector.tensor_tensor(out=cnt[:], in0=stb_f[:], in1=sta_f[:],
                                    op=mybir.AluOpType.subtract)
            nc.vector.tensor_scalar(out=cnt[:], in0=cnt[:], scalar1=1.0,
                                    scalar2=None, op0=mybir.AluOpType.max)
            recip = cst.tile([P, NGRP], F32, name="recip")
            nc.vector.reciprocal(out=recip[:], in_=cnt[:])

            # ---- software pipeline over 16 groups ----
            # per step g: compares (if new quarter), emb matmuls g, scan g,
            # gather g-1, diff g-2, proj g-3, scale-copy g-3, out dma g-3
            # (diff lags the gather by one step so the gpsimd gather's launch
            # latency overlaps the next scan instead of stalling DVE)
            # pre-zero col0 of the scg ring buffers (scan writes [:, 1:] only)
            for _i in range(3):
                sc0 = work.tile([P, SL], F32, name="scg", tag="scg", bufs=3)
                nc.vector.memset(sc0[:, 0:1], 0.0)

            ohs = [None] * NQ          # live one-hot tiles per quarter
            psg = [None] * NGRP
            scg = [None] * NGRP
            bnd = [None] * NGRP
            grp = [None] * NGRP
            pso = [None] * NGRP
            osb = [None] * NGRP

            def emit_compares(q):
                lo, hi = qwin[q]
                tiles = []
                for v in range(V // P):
                    oh = oh_pool.tile([P, hi - lo], BF16, name=f"oh{v}",
                                      tag=f"oh{v}", bufs=2)
                    nc.vector.tensor_scalar(
                        out=oh[:], in0=xq[q][:], scalar1=vcol_t[:, v:v + 1],
                        scalar2=None, op0=mybir.AluOpType.is_equal)
                    tiles.append(oh)
                ohs[q] = tiles

            def emit_emb(g):
                q = g // GPQ
                qlo = qwin[q][0]
                lo, hi = windows[g]
                L = hi - lo
                pg = ps_g.tile([P, WCAP], F32, name="psg", tag="psg", bufs=3)
                nsub = (L + 511) // 512
                for s in range(nsub):
                    c0, c1 = 512 * s, min(512 * (s + 1), L)
                    for v in range(V // P):
                        nc.tensor.matmul(
                            out=pg[:, c0:c1], lhsT=embb[v][:],
                            rhs=ohs[q][v][:, lo - qlo + c0:lo - qlo + c1],
                            start=(v == 0), stop=(v == V // P - 1))
                psg[g] = pg

            def emit_scan(g):
                L = windows[g][1] - windows[g][0]
                sc = work.tile([P, SL], F32, name="scg", tag="scg", bufs=3)
                nc.vector.tensor_tensor_scan(
                    out=sc[:, 1:1 + L], data0=psg[g][:, 0:L],
                    data1=zcol[:].to_broadcast([P, L]),
                    initial=0.0, op0=ADD, op1=ADD)
                scg[g] = sc

            def emit_gather(g):
                bt = work.tile([P, NBPAD], F32, name="bnd", tag="bnd", bufs=4)
                nc.gpsimd.ap_gather(
                    out_ap=bt[:], in_ap=scg[g][:],
                    idxs_ap=bidx_t[:, g * NBSLOT:g * NBSLOT + NBPAD // 16],
                    channels=P, num_elems=SL, d=1, num_idxs=NBPAD)
                bnd[g] = bt

            def emit_diff(g):
                gt = work.tile([P, TG], BF16, name="grp", tag="grp", bufs=3)
                nc.vector.tensor_tensor(
                    out=gt[:], in0=bnd[g][:, 1:NB], in1=bnd[g][:, 0:NB - 1],
                    op=mybir.AluOpType.subtract)
                grp[g] = gt

            def emit_proj(g):
                halves = []
                for h in range(2):
                    po = ps_o.tile([P, E // 2], F32, name="pso", tag="pso", bufs=2)
                    nc.tensor.matmul(
                        out=po[:], lhsT=grp[g][:],
                        rhs=wtb[:, h * 512:(h + 1) * 512],
                        start=True, stop=True)
                    halves.append(po)
                pso[g] = halves

            def emit_out(g):
                ot = work.tile([P, E], F32, name="osb", tag="osb", bufs=3)
                for h in range(2):
                    nc.scalar.mul(ot[:, h * 512:(h + 1) * 512], pso[g][h][:],
                                  recip[:, g:g + 1])
                nc.sync.dma_start(out=out.ap()[g * TG:(g + 1) * TG, :], in_=ot[:])
                osb[g] = ot

            for step in range(NGRP + 4):
                g = step
                if 4 <= g <= NGRP + 3:
                    emit_proj(g - 4)
                if g < NGRP:
                    if g % GPQ == 0:
                        emit_compares(g // GPQ)
                    emit_emb(g)
                    emit_scan(g)
                if 1 <= g <= NGRP:
                    emit_gather(g - 1)
                if 3 <= g <= NGRP + 2:
                    emit_diff(g - 3)
                if 4 <= g <= NGRP + 3:
                    emit_out(g - 4)

    nc.compile()
    return nc


def _prep_inputs(x, byte_groups, emb_weight, out_proj_w, windows, starts):
    """Host-side integer index plumbing + weight layout prep."""
    wt_np = np.ascontiguousarray(np.asarray(out_proj_w, np.float32).T)  # [128,1024]
    emb_np = np.ascontiguousarray(np.asarray(emb_weight, np.float32))
    vcol_np = np.zeros((P, V // P), np.float32)
    for v in range(V // P):
        vcol_np[:, v] = v * P + np.arange(P)

    in_maps = []
    for k in range(B):
        sta = starts[k, :T].reshape(NGRP, TG).transpose(1, 0).astype(np.int32)
        stb = starts[k, 1:T + 1].reshape(NGRP, TG).transpose(1, 0).astype(np.int32)
        # boundary indices per group, wrapped in 16 partitions, x8 replicated
        bx = np.zeros((P, NGRP * NBSLOT), np.int16)
        for g in range(NGRP):
            lo = windows[g][0]
            loc = (starts[k, TG * g:TG * (g + 1) + 1] - lo).astype(np.int16)
            pad = np.full(NBPAD, loc[-1], np.int16)
            pad[:NB] = loc
            w = pad.reshape(NBPAD // 16, 16).T  # [16, 9]
            for rep in range(8):
                bx[16 * rep:16 * (rep + 1),
                   g * NBSLOT:g * NBSLOT + NBPAD // 16] = w
        in_maps.append({
            "x1": x[k].astype(np.int16).reshape(1, S),
            "st_a": np.ascontiguousarray(sta),
            "st_b": np.ascontiguousarray(stb),
            "bidx": bx,
            "vcol": vcol_np,
            "emb_weight": emb_np,
            "wt": wt_np,
        })
    return in_maps


def _run(x, byte_groups, emb_weight, out_proj_w, trace=False, **kw):
    x = np.asarray(x)
    byte_groups = np.asarray(byte_groups)
    starts = np.stack(
        [np.searchsorted(byte_groups[k], np.arange(T + 1)) for k in range(B)]
    )
    windows = _windows(starts)
    assert max(hi - lo for lo, hi in windows) <= WCAP, windows
    nc = _build(windows)
    in_maps = _prep_inputs(x, byte_groups, emb_weight, out_proj_w, windows, starts)
    res = run_bass_kernel_spmd(nc, in_maps, core_ids=list(range(B)), trace=trace, **kw)
    outs = np.stack([res.results[k]["out"] for k in range(B)], axis=0)
    return outs, res


def kernel(x, byte_groups, emb_weight, out_proj_w):
    outs, _ = _run(x, byte_groups, emb_weight, out_proj_w, trace=False)
    return outs


# revision 14
# speedup vs baseline: 1.1895x; 1.0089x over previous
"""Trainium2 Bass kernel for nn_ByteEmbedding (segment_reduce).

Computation (per batch row, one row per NeuronCore, 8 cores):
  byte_emb = emb_weight[x] * sqrt(128)            # gather  [8192, 128]
  grouped  = segment_mean(byte_emb, byte_groups)  # ragged  [2048, 128]
  out      = grouped @ out_proj_w.T               # proj    [2048, 1024]

v2 pipeline (transposed layout [dim, pos], 16 groups of 128 tokens):
  1. x uploaded once as [1, S] int16; partition-broadcast DMA replicates it
     to [128, W] per quarter-window (16 KB HBM traffic instead of 2 MB).
  2. One-hot vocab rows XohT[v, i] = (x[i] == v) built on DVE in bf16
     (int16-in/bf16-out hits the 4x DVE mode; 3 chunks of 128 vocab rows,
     one set per quarter-window).
  3. byte_emb^T = (E*sqrt(128))^T @ XohT as bf16 matmuls on the PE
     (1 cycle/row vs 4 for fp32), accumulated per group window in PSUM.
  4. Exclusive prefix sums per group window (DVE tensor_tensor_scan, f32,
     read straight from PSUM); segment sums are differences of the scan at
     host-precomputed boundary positions (gpsimd ap_gather).
  5. Mean via reciprocal counts folded into the mandatory PSUM->SBUF copy
     after the projection (ACT scalar.mul with per-partition scale).
  6. out = grp^T.T @ W^T per 128-token group in bf16; W^T is uploaded
     host-transposed and cast to bf16 on device.
  7. Output DMAs alternate between the SP and Activation HW queues.

The group windows are the union over the 8 rows, so one SPMD program
serves all cores; per-core behavior enters only through uploaded integer
index tensors.
"""

import os
import sys

import numpy as np

for _p in ("/opt/trn_rl_repo",):
    if _p not in sys.path and os.path.isdir(_p):
        sys.path.append(_p)

import concourse.bacc as bacc
import concourse.bass as bass
import concourse.mybir as mybir
import concourse.tile as tile
from concourse.bass_utils import run_bass_kernel_spmd

B = 8
S = 8192          # bytes per row
V = 384           # vocab (= 3 * 128)
D = 128           # byte dim
E = 1024          # out dim
T = 2048          # tokens
P = 128
NGRP = 16         # token groups of 128
TG = T // NGRP    # 128 tokens per group
NB = TG + 1       # boundaries per group (inclusive)
NBPAD = 144       # padded boundary count (16*9, %4==0)
NBSLOT = 16       # idx words per group slot (32-byte aligned for ap_gather)
WCAP = 1024       # max positions per group window (psum: 2 banks)
SL = WCAP + 1     # scan tile length
SCALE = float(D) ** 0.5
dt = mybir.dt
F32 = dt.float32
BF16 = dt.bfloat16
ADD = mybir.AluOpType.add


def _windows(starts):
    """Union [lo, hi) position window per group / quarter over all rows."""
    w = []
    for g in range(NGRP):
        lo = int(starts[:, TG * g].min())
        hi = int(starts[:, TG * (g + 1)].max())
        w.append((lo, hi))
    return w


def _build(windows) -> bacc.Bacc:
    nc = bacc.Bacc(
        "TRN2",
        target_bir_lowering=False,
        debug=False,
        enable_asserts=True,
        num_devices=B,
    )

    x1 = nc.dram_tensor("x1", [1, S], dt.int16, kind="ExternalInput")
    st_a = nc.dram_tensor("st_a", [P, NGRP], dt.int32, kind="ExternalInput")
    st_b = nc.dram_tensor("st_b", [P, NGRP], dt.int32, kind="ExternalInput")
    bidx = nc.dram_tensor("bidx", [P, NGRP * NBSLOT], dt.int16,
                          kind="ExternalInput")
    vcol = nc.dram_tensor("vcol", [P, V // P], F32, kind="ExternalInput")
    emb_weight = nc.dram_tensor("emb_weight", [V, D], F32, kind="ExternalInput")
    wt = nc.dram_tensor("wt", [D, E], F32, kind="ExternalInput")  # host-transposed
    out = nc.dram_tensor("out", [T, E], F32, kind="ExternalOutput")

    # quarter compare-windows (4 groups each)
    NQ = 4
    GPQ = NGRP // NQ
    qwin = []
    for q in range(NQ):
        qwin.append((windows[GPQ * q][0], windows[GPQ * q + GPQ - 1][1]))

    with tile.TileContext(nc) as tc:
        with (
            tc.tile_pool(name="cst", bufs=1) as cst,
            tc.tile_pool(name="xq", bufs=1) as xq_pool,
            tc.tile_pool(name="oh", bufs=1) as oh_pool,
            tc.tile_pool(name="work", bufs=1) as work,
            tc.tile_pool(name="ps_g", bufs=1, space="PSUM") as ps_g,
            tc.tile_pool(name="ps_o", bufs=1, space="PSUM") as ps_o,
        ):
            # vcol first on the fast Sync queue: it gates the first compare
            vcol_t = cst.tile([P, V // P], F32, name="vcol_t")
            nc.sync.dma_start(out=vcol_t[:], in_=vcol.ap())
            bidx_t = cst.tile([P, NGRP * NBSLOT], dt.int16, name="bidx_t")
            nc.sync.dma_start(out=bidx_t[:], in_=bidx.ap())
            sta_i = cst.tile([P, NGRP], dt.int32, name="sta_i")
            nc.sync.dma_start(out=sta_i[:], in_=st_a.ap())
            stb_i = cst.tile([P, NGRP], dt.int32, name="stb_i")
            nc.sync.dma_start(out=stb_i[:], in_=st_b.ap())

            # ---- x quarter-window broadcasts (Sync queue) ----
            xq = []
            for q in range(NQ):
                lo, hi = qwin[q]
                xt = xq_pool.tile([P, hi - lo], dt.int16, name=f"xq{q}",
                                  tag="xq", bufs=NQ)
                nc.sync.dma_start(out=xt[:], in_=x1.ap()[0:1, lo:hi].to_broadcast(
                    [P, hi - lo]))
                xq.append(xt)

            # ---- prologue: constants / weights ----
            zcol = cst.tile([P, 1], F32, name="zcol")
            nc.vector.memset(zcol[:], 0.0)

            # emb chunks + bf16 prep on ACT (needed by first emb matmul)
            emb_f = []
            for v in range(V // P):
                ef = cst.tile([P, D], F32, name=f"emb_f{v}")
                nc.scalar.dma_start(out=ef[:], in_=emb_weight.ap()[v * P:(v + 1) * P, :])
                emb_f.append(ef)
            embb = []
            for v in range(V // P):
                eb = cst.tile([P, D], BF16, name=f"embb{v}")
                nc.scalar.mul(eb[:], emb_f[v][:], SCALE)
                embb.append(eb)

            wt_f = cst.tile([P, E], F32, name="wt_f")
            nc.scalar.dma_start(out=wt_f[:], in_=wt.ap())
            wtb = cst.tile([P, E], BF16, name="wtb")
            nc.scalar.copy(wtb[:], wt_f[:])

            # recip counts computed later (emitted after first compares so the
            # DVE queue is not blocked at t=0 waiting for the st_a/st_b DMAs)
            recip = cst.tile([P, NGRP], F32, name="recip")

            def emit_recip():
                sta_f = cst.tile([P, NGRP], F32, name="sta_f")
                nc.vector.tensor_copy(out=sta_f[:], in_=sta_i[:])
                stb_f = cst.tile([P, NGRP], F32, name="stb_f")
                nc.vector.tensor_copy(out=stb_f[:], in_=stb_i[:])
                cnt = cst.tile([P, NGRP], F32, name="cnt")
                nc.vector.tensor_tensor(out=cnt[:], in0=stb_f[:], in1=sta_f[:],
                                        op=mybir.AluOpType.subtract)
                nc.vector.tensor_scalar(out=cnt[:], in0=cnt[:], scalar1=1.0,
                                        scalar2=None, op0=mybir.AluOpType.max)
                nc.vector.reciprocal(out=recip[:], in_=cnt[:])

            # ---- software pipeline over 16 groups ----
            # per step g: compares (if new quarter), emb matmuls g, scan g,
            # gather g-1, diff g-2, proj g-3, scale-copy g-3, out dma g-3
            # (diff lags the gather by one step so the gpsimd gather's launch
            # latency overlaps the next scan instead of stalling DVE)
            # pre-zero col0 of the scg ring buffers (scan writes [:, 1:] only)
            for _i in range(3):
                sc0 = work.tile([P, SL], F32, name="scg", tag="scg", bufs=3)
                nc.vector.memset(sc0[:, 0:1], 0.0)

            ohs = [None] * NQ          # live one-hot tiles per quarter
            psg = [None] * NGRP
            scg = [None] * NGRP
            bnd = [None] * NGRP
            grp = [None] * NGRP
            pso = [None] * NGRP
            osb = [None] * NGRP

            def emit_compares(q):
                lo, hi = qwin[q]
                tiles = []
                for v in range(V // P):
                    oh = oh_pool.tile([P, hi - lo], BF16, name=f"oh{v}",
                                      tag=f"oh{v}", bufs=2)
                    nc.vector.tensor_scalar(
                        out=oh[:], in0=xq[q][:], scalar1=vcol_t[:, v:v + 1],
                        scalar2=None, op0=mybir.AluOpType.is_equal)
                    tiles.append(oh)
                ohs[q] = tiles

            def emit_emb(g):
                q = g // GPQ
                qlo = qwin[q][0]
                lo, hi = windows[g]
                L = hi - lo
                pg = ps_g.tile([P, WCAP], F32, name="psg", tag="psg", bufs=3)
                nsub = (L + 511) // 512
                for s in range(nsub):
                    c0, c1 = 512 * s, min(512 * (s + 1), L)
                    for v in range(V // P):
                        nc.tensor.matmul(
                            out=pg[:, c0:c1], lhsT=embb[v][:],
                            rhs=ohs[q][v][:, lo - qlo + c0:lo - qlo + c1],
                            start=(v == 0), stop=(v == V // P - 1))
                psg[g] = pg

            def emit_scan(g):
                L = windows[g][1] - windows[g][0]
                sc = work.tile([P, SL], F32, name="scg", tag="scg", bufs=3)
                nc.vector.tensor_tensor_scan(
                    out=sc[:, 1:1 + L], data0=psg[g][:, 0:L],
                    data1=zcol[:].to_broadcast([P, L]),
                    initial=0.0, op0=ADD, op1=ADD)
                scg[g] = sc

            def emit_gather(g):
                bt = work.tile([P, NBPAD], F32, name="bnd", tag="bnd", bufs=4)
                nc.gpsimd.ap_gather(
                    out_ap=bt[:], in_ap=scg[g][:],
                    idxs_ap=bidx_t[:, g * NBSLOT:g * NBSLOT + NBPAD // 16],
                    channels=P, num_elems=SL, d=1, num_idxs=NBPAD)
                bnd[g] = bt

            def emit_diff(g):
                gt = work.tile([P, TG], BF16, name="grp", tag="grp", bufs=3)
                nc.vector.tensor_tensor(
                    out=gt[:], in0=bnd[g][:, 1:NB], in1=bnd[g][:, 0:NB - 1],
                    op=mybir.AluOpType.subtract)
                grp[g] = gt

            def emit_proj(g):
                halves = []
                for h in range(2):
                    po = ps_o.tile([P, E // 2], F32, name="pso", tag="pso", bufs=2)
                    nc.tensor.matmul(
                        out=po[:], lhsT=grp[g][:],
                        rhs=wtb[:, h * 512:(h + 1) * 512],
                        start=True, stop=True)
                    halves.append(po)
                pso[g] = halves

            def emit_out(g):
                ot = work.tile([P, E], F32, name="osb", tag="osb", bufs=3)
                for h in range(2):
                    nc.scalar.mul(ot[:, h * 512:(h + 1) * 512], pso[g][h][:],
                                  recip[:, g:g + 1])
                nc.sync.dma_start(out=out.ap()[g * TG:(g + 1) * TG, :], in_=ot[:])
                osb[g] = ot

            for step in range(NGRP + 4):
                g = step
                if 4 <= g <= NGRP + 3:
                    emit_proj(g - 4)
                if g < NGRP:
                    if g % GPQ == 0:
                        emit_compares(g // GPQ)
                    if g == 0:
                        emit_recip()
                    emit_emb(g)
                    emit_scan(g)
                if 1 <= g <= NGRP:
                    emit_gather(g - 1)
                if 3 <= g <= NGRP + 2:
                    emit_diff(g - 3)
                if 4 <= g <= NGRP + 3:
                    emit_out(g - 4)

    nc.compile()
    return nc


def _prep_inputs(x, byte_groups, emb_weight, out_proj_w, windows, starts):
    """Host-side integer index plumbing + weight layout prep."""
    wt_np = np.ascontiguousarray(np.asarray(out_proj_w, np.float32).T)  # [128,1024]
    emb_np = np.ascontiguousarray(np.asarray(emb_weight, np.float32))
    vcol_np = np.zeros((P, V // P), np.float32)
    for v in range(V // P):
        vcol_np[:, v] = v * P + np.arange(P)

    in_maps = []
    for k in range(B):
        sta = starts[k, :T].reshape(NGRP, TG).transpose(1, 0).astype(np.int32)
        stb = starts[k, 1:T + 1].reshape(NGRP, TG).transpose(1, 0).astype(np.int32)
        # boundary indices per group, wrapped in 16 partitions, x8 replicated
        bx = np.zeros((P, NGRP * NBSLOT), np.int16)
        for g in range(NGRP):
            lo = windows[g][0]
            loc = (starts[k, TG * g:TG * (g + 1) + 1] - lo).astype(np.int16)
            pad = np.full(NBPAD, loc[-1], np.int16)
            pad[:NB] = loc
            w = pad.reshape(NBPAD // 16, 16).T  # [16, 9]
            for rep in range(8):
                bx[16 * rep:16 * (rep + 1),
                   g * NBSLOT:g * NBSLOT + NBPAD // 16] = w
        in_maps.append({
            "x1": x[k].astype(np.int16).reshape(1, S),
            "st_a": np.ascontiguousarray(sta),
            "st_b": np.ascontiguousarray(stb),
            "bidx": bx,
            "vcol": vcol_np,
            "emb_weight": emb_np,
            "wt": wt_np,
        })
    return in_maps


def _run(x, byte_groups, emb_weight, out_proj_w, trace=False, **kw):
    x = np.asarray(x)
    byte_groups = np.asarray(byte_groups)
    starts = np.stack(
        [np.searchsorted(byte_groups[k], np.arange(T + 1)) for k in range(B)]
    )
    windows = _windows(starts)
    assert max(hi - lo for lo, hi in windows) <= WCAP, windows
    nc = _build(windows)
    in_maps = _prep_inputs(x, byte_groups, emb_weight, out_proj_w, windows, starts)
    res = run_bass_kernel_spmd(nc, in_maps, core_ids=list(range(B)), trace=trace, **kw)
    outs = np.stack([res.results[k]["out"] for k in range(B)], axis=0)
    return outs, res


def kernel(x, byte_groups, emb_weight, out_proj_w):
    outs, _ = _run(x, byte_groups, emb_weight, out_proj_w, trace=False)
    return outs


# revision 15
# speedup vs baseline: 1.1960x; 1.0054x over previous
"""Trainium2 Bass kernel for nn_ByteEmbedding (segment_reduce).

Computation (per batch row, one row per NeuronCore, 8 cores):
  byte_emb = emb_weight[x] * sqrt(128)            # gather  [8192, 128]
  grouped  = segment_mean(byte_emb, byte_groups)  # ragged  [2048, 128]
  out      = grouped @ out_proj_w.T               # proj    [2048, 1024]

v2 pipeline (transposed layout [dim, pos], 16 groups of 128 tokens):
  1. x uploaded once as [1, S] int16; partition-broadcast DMA replicates it
     to [128, W] per quarter-window (16 KB HBM traffic instead of 2 MB).
  2. One-hot vocab rows XohT[v, i] = (x[i] == v) built on DVE in bf16
     (int16-in/bf16-out hits the 4x DVE mode; 3 chunks of 128 vocab rows,
     one set per quarter-window).
  3. byte_emb^T = (E*sqrt(128))^T @ XohT as bf16 matmuls on the PE
     (1 cycle/row vs 4 for fp32), accumulated per group window in PSUM.
  4. Exclusive prefix sums per group window (DVE tensor_tensor_scan, f32,
     read straight from PSUM); segment sums are differences of the scan at
     host-precomputed boundary positions (gpsimd ap_gather).
  5. Mean via reciprocal counts folded into the mandatory PSUM->SBUF copy
     after the projection (ACT scalar.mul with per-partition scale).
  6. out = grp^T.T @ W^T per 128-token group in bf16; W^T is uploaded
     host-transposed and cast to bf16 on device.
  7. Output DMAs alternate between the SP and Activation HW queues.

The group windows are the union over the 8 rows, so one SPMD program
serves all cores; per-core behavior enters only through uploaded integer
index tensors.
"""

import os
import sys

import numpy as np

for _p in ("/opt/trn_rl_repo",):
    if _p not in sys.path and os.path.isdir(_p):
        sys.path.append(_p)

import concourse.bacc as bacc
import concourse.bass as bass
import concourse.mybir as mybir
import concourse.tile as tile
from concourse.bass_utils import run_bass_kernel_spmd

B = 8
S = 8192          # bytes per row
V = 384           # vocab (= 3 * 128)
D = 128           # byte dim
E = 1024          # out dim
T = 2048          # tokens
P = 128
NGRP = 16         # token groups of 128
TG = T // NGRP    # 128 tokens per group
NB = TG + 1       # boundaries per group (inclusive)
NBPAD = 144       # padded boundary count (16*9, %4==0)
NBSLOT = 16       # idx words per group slot (32-byte aligned for ap_gather)
WCAP = 1024       # max positions per group window (psum: 2 banks)
SL = WCAP + 1     # scan tile length
SCALE = float(D) ** 0.5
dt = mybir.dt
F32 = dt.float32
BF16 = dt.bfloat16
ADD = mybir.AluOpType.add


def _windows(starts):
    """Union [lo, hi) position window per group / quarter over all rows."""
    w = []
    for g in range(NGRP):
        lo = int(starts[:, TG * g].min())
        hi = int(starts[:, TG * (g + 1)].max())
        w.append((lo, hi))
    return w


def _build(windows) -> bacc.Bacc:
    nc = bacc.Bacc(
        "TRN2",
        target_bir_lowering=False,
        debug=False,
        enable_asserts=False,
        num_devices=B,
    )

    x1 = nc.dram_tensor("x1", [1, S], dt.int16, kind="ExternalInput")
    st_a = nc.dram_tensor("st_a", [P, NGRP], dt.int32, kind="ExternalInput")
    st_b = nc.dram_tensor("st_b", [P, NGRP], dt.int32, kind="ExternalInput")
    bidx = nc.dram_tensor("bidx", [P, NGRP * NBSLOT], dt.int16,
                          kind="ExternalInput")
    vcol = nc.dram_tensor("vcol", [P, V // P], F32, kind="ExternalInput")
    emb_weight = nc.dram_tensor("emb_weight", [V, D], F32, kind="ExternalInput")
    wt = nc.dram_tensor("wt", [D, E], F32, kind="ExternalInput")  # host-transposed
    out = nc.dram_tensor("out", [T, E], F32, kind="ExternalOutput")

    # quarter compare-windows (4 groups each)
    NQ = 4
    GPQ = NGRP // NQ
    qwin = []
    for q in range(NQ):
        qwin.append((windows[GPQ * q][0], windows[GPQ * q + GPQ - 1][1]))

    with tile.TileContext(nc) as tc:
        with (
            tc.tile_pool(name="cst", bufs=1) as cst,
            tc.tile_pool(name="xq", bufs=1) as xq_pool,
            tc.tile_pool(name="oh", bufs=1) as oh_pool,
            tc.tile_pool(name="work", bufs=1) as work,
            tc.tile_pool(name="ps_g", bufs=1, space="PSUM") as ps_g,
            tc.tile_pool(name="ps_o", bufs=1, space="PSUM") as ps_o,
        ):
            # vcol first on the fast Sync queue: it gates the first compare
            vcol_t = cst.tile([P, V // P], F32, name="vcol_t")
            nc.sync.dma_start(out=vcol_t[:], in_=vcol.ap())
            bidx_t = cst.tile([P, NGRP * NBSLOT], dt.int16, name="bidx_t")
            nc.sync.dma_start(out=bidx_t[:], in_=bidx.ap())
            sta_i = cst.tile([P, NGRP], dt.int32, name="sta_i")
            nc.sync.dma_start(out=sta_i[:], in_=st_a.ap())
            stb_i = cst.tile([P, NGRP], dt.int32, name="stb_i")
            nc.sync.dma_start(out=stb_i[:], in_=st_b.ap())

            # ---- x quarter-window broadcasts (Sync queue) ----
            xq = []
            for q in range(NQ):
                lo, hi = qwin[q]
                xt = xq_pool.tile([P, hi - lo], dt.int16, name=f"xq{q}",
                                  tag="xq", bufs=NQ)
                nc.sync.dma_start(out=xt[:], in_=x1.ap()[0:1, lo:hi].to_broadcast(
                    [P, hi - lo]))
                xq.append(xt)

            # ---- prologue: constants / weights ----
            zcol = cst.tile([P, 1], F32, name="zcol")
            nc.vector.memset(zcol[:], 0.0)

            # emb chunks + bf16 prep on ACT (needed by first emb matmul)
            emb_f = []
            for v in range(V // P):
                ef = cst.tile([P, D], F32, name=f"emb_f{v}")
                nc.scalar.dma_start(out=ef[:], in_=emb_weight.ap()[v * P:(v + 1) * P, :])
                emb_f.append(ef)
            embb = []
            for v in range(V // P):
                eb = cst.tile([P, D], BF16, name=f"embb{v}")
                nc.scalar.mul(eb[:], emb_f[v][:], SCALE)
                embb.append(eb)

            wt_f = cst.tile([P, E], F32, name="wt_f")
            nc.scalar.dma_start(out=wt_f[:], in_=wt.ap())
            wtb = cst.tile([P, E], BF16, name="wtb")
            nc.scalar.copy(wtb[:], wt_f[:])

            # recip counts computed later (emitted after first compares so the
            # DVE queue is not blocked at t=0 waiting for the st_a/st_b DMAs)
            recip = cst.tile([P, NGRP], F32, name="recip")

            def emit_recip():
                sta_f = cst.tile([P, NGRP], F32, name="sta_f")
                nc.vector.tensor_copy(out=sta_f[:], in_=sta_i[:])
                stb_f = cst.tile([P, NGRP], F32, name="stb_f")
                nc.vector.tensor_copy(out=stb_f[:], in_=stb_i[:])
                cnt = cst.tile([P, NGRP], F32, name="cnt")
                nc.vector.tensor_tensor(out=cnt[:], in0=stb_f[:], in1=sta_f[:],
                                        op=mybir.AluOpType.subtract)
                nc.vector.tensor_scalar(out=cnt[:], in0=cnt[:], scalar1=1.0,
                                        scalar2=None, op0=mybir.AluOpType.max)
                nc.vector.reciprocal(out=recip[:], in_=cnt[:])

            # ---- software pipeline over 16 groups ----
            # per step g: compares (if new quarter), emb matmuls g, scan g,
            # gather g-1, diff g-2, proj g-3, scale-copy g-3, out dma g-3
            # (diff lags the gather by one step so the gpsimd gather's launch
            # latency overlaps the next scan instead of stalling DVE)
            # pre-zero col0 of the scg ring buffers (scan writes [:, 1:] only)
            for _i in range(3):
                sc0 = work.tile([P, SL], F32, name="scg", tag="scg", bufs=3)
                nc.vector.memset(sc0[:, 0:1], 0.0)

            ohs = [None] * NQ          # live one-hot tiles per quarter
            psg = [None] * NGRP
            scg = [None] * NGRP
            bnd = [None] * NGRP
            grp = [None] * NGRP
            pso = [None] * NGRP
            osb = [None] * NGRP

            def emit_compares(q):
                lo, hi = qwin[q]
                tiles = []
                for v in range(V // P):
                    oh = oh_pool.tile([P, hi - lo], BF16, name=f"oh{v}",
                                      tag=f"oh{v}", bufs=2)
                    nc.vector.tensor_scalar(
                        out=oh[:], in0=xq[q][:], scalar1=vcol_t[:, v:v + 1],
                        scalar2=None, op0=mybir.AluOpType.is_equal)
                    tiles.append(oh)
                ohs[q] = tiles

            def emit_emb(g):
                q = g // GPQ
                qlo = qwin[q][0]
                lo, hi = windows[g]
                L = hi - lo
                pg = ps_g.tile([P, WCAP], F32, name="psg", tag="psg", bufs=3)
                nsub = (L + 511) // 512
                for s in range(nsub):
                    c0, c1 = 512 * s, min(512 * (s + 1), L)
                    for v in range(V // P):
                        nc.tensor.matmul(
                            out=pg[:, c0:c1], lhsT=embb[v][:],
                            rhs=ohs[q][v][:, lo - qlo + c0:lo - qlo + c1],
                            start=(v == 0), stop=(v == V // P - 1))
                psg[g] = pg

            def emit_scan(g):
                L = windows[g][1] - windows[g][0]
                sc = work.tile([P, SL], F32, name="scg", tag="scg", bufs=3)
                nc.vector.tensor_tensor_scan(
                    out=sc[:, 1:1 + L], data0=psg[g][:, 0:L],
                    data1=zcol[:].to_broadcast([P, L]),
                    initial=0.0, op0=ADD, op1=ADD)
                scg[g] = sc

            def emit_gather(g):
                bt = work.tile([P, NBPAD], F32, name="bnd", tag="bnd", bufs=4)
                nc.gpsimd.ap_gather(
                    out_ap=bt[:], in_ap=scg[g][:],
                    idxs_ap=bidx_t[:, g * NBSLOT:g * NBSLOT + NBPAD // 16],
                    channels=P, num_elems=SL, d=1, num_idxs=NBPAD)
                bnd[g] = bt

            def emit_diff(g):
                gt = work.tile([P, TG], BF16, name="grp", tag="grp", bufs=3)
                nc.vector.tensor_tensor(
                    out=gt[:], in0=bnd[g][:, 1:NB], in1=bnd[g][:, 0:NB - 1],
                    op=mybir.AluOpType.subtract)
                grp[g] = gt

            def emit_proj(g):
                halves = []
                for h in range(2):
                    po = ps_o.tile([P, E // 2], F32, name="pso", tag="pso", bufs=2)
                    nc.tensor.matmul(
                        out=po[:], lhsT=grp[g][:],
                        rhs=wtb[:, h * 512:(h + 1) * 512],
                        start=True, stop=True)
                    halves.append(po)
                pso[g] = halves

            def emit_out(g):
                ot = work.tile([P, E], F32, name="osb", tag="osb", bufs=3)
                for h in range(2):
                    nc.scalar.mul(ot[:, h * 512:(h + 1) * 512], pso[g][h][:],
                                  recip[:, g:g + 1])
                nc.sync.dma_start(out=out.ap()[g * TG:(g + 1) * TG, :], in_=ot[:])
                osb[g] = ot

            for step in range(NGRP + 4):
                g = step
                if 4 <= g <= NGRP + 3:
                    emit_proj(g - 4)
                if g < NGRP:
                    if g % GPQ == 0:
                        emit_compares(g // GPQ)
                    if g == 0:
                        emit_recip()
                    emit_emb(g)
                    emit_scan(g)
                if 1 <= g <= NGRP:
                    emit_gather(g - 1)
                if 3 <= g <= NGRP + 2:
                    emit_diff(g - 3)
                if 4 <= g <= NGRP + 3:
                    emit_out(g - 4)

    nc.compile()
    return nc


def _prep_inputs(x, byte_groups, emb_weight, out_proj_w, windows, starts):
    """Host-side integer index plumbing + weight layout prep."""
    wt_np = np.ascontiguousarray(np.asarray(out_proj_w, np.float32).T)  # [128,1024]
    emb_np = np.ascontiguousarray(np.asarray(emb_weight, np.float32))
    vcol_np = np.zeros((P, V // P), np.float32)
    for v in range(V // P):
        vcol_np[:, v] = v * P + np.arange(P)

    in_maps = []
    for k in range(B):
        sta = starts[k, :T].reshape(NGRP, TG).transpose(1, 0).astype(np.int32)
        stb = starts[k, 1:T + 1].reshape(NGRP, TG).transpose(1, 0).astype(np.int32)
        # boundary indices per group, wrapped in 16 partitions, x8 replicated
        bx = np.zeros((P, NGRP * NBSLOT), np.int16)
        for g in range(NGRP):
            lo = windows[g][0]
            loc = (starts[k, TG * g:TG * (g + 1) + 1] - lo).astype(np.int16)
            pad = np.full(NBPAD, loc[-1], np.int16)
            pad[:NB] = loc
            w = pad.reshape(NBPAD // 16, 16).T  # [16, 9]
            for rep in range(8):
                bx[16 * rep:16 * (rep + 1),
                   g * NBSLOT:g * NBSLOT + NBPAD // 16] = w
        in_maps.append({
            "x1": x[k].astype(np.int16).reshape(1, S),
            "st_a": np.ascontiguousarray(sta),
            "st_b": np.ascontiguousarray(stb),
            "bidx": bx,
            "vcol": vcol_np,
            "emb_weight": emb_np,
            "wt": wt_np,
        })
    return in_maps


def _run(x, byte_groups, emb_weight, out_proj_w, trace=False, **kw):
    x = np.asarray(x)
    byte_groups = np.asarray(byte_groups)
    starts = np.stack(
        [np.searchsorted(byte_groups[k], np.arange(T + 1)) for k in range(B)]
    )
    windows = _windows(starts)
    assert max(hi - lo for lo, hi in windows) <= WCAP, windows
    nc = _build(windows)
    in_maps = _prep_inputs(x, byte_groups, emb_weight, out_proj_w, windows, starts)
    res = run_bass_kernel_spmd(nc, in_maps, core_ids=list(range(B)), trace=trace, **kw)
    outs = np.stack([res.results[k]["out"] for k in range(B)], axis=0)
    return outs, res


def kernel(x, byte_groups, emb_weight, out_proj_w):
    outs, _ = _run(x, byte_groups, emb_weight, out_proj_w, trace=False)
    return outs
